# revision 1
# baseline (speedup 1.0000x reference)
"""Trainium2 Bass kernel for nn_DecoderRNN: 19-step greedy LSTM decode.

Strategy (8 NeuronCores, SPMD single NEFF):
- Vocab-parallel GEMV: W_lin/b_lin sharded by vocab rows across cores; each
  core keeps most of its W_lin.T shard resident in SBUF as float32r
  (fp32 with 12 mantissa bits dropped, 1 PE cycle/row) and streams the rest
  from HBM per step. logits tile [1,512] per matmul group, bias folded in via
  a K=1 matmul. Per-tile top-1 (max8/max_index on DVE) -> per-core winner
  -> tiny AllGather -> global argmax computed redundantly on every core.
- LSTM contraction-parallel: W_ih/W_hh sharded by the contraction dim
  (x-dims 64/core, h-dims 128/core); per-core partial pre-activations [4096]
  are AllReduce-summed, then every core computes the full cell elementwise
  (gates laid out [128, 32] partition-major).
- Embedding row fetch: per-core indirect DMA gather of emb[tok, 64k:64k+64]
  using per-partition offsets; no communication.

float32r numerics were validated host-side: RNE-rounding the low 12 mantissa
bits of all weights/h/x reproduces the exact reference token sequence.
"""

import os

import numpy as np

import concourse.bass as bass
import concourse.mybir as mybir
import concourse.tile as tile
from concourse.bass import IndirectOffsetOnAxis
from concourse.bass_utils import run_bass_kernel_spmd

E, H, V = 512, 1024, 50257
NCORES = 8
STEPS = int(os.environ.get("KSTEPS", "19"))
VSH = 6283  # vocab rows per core (8*6283 >= 50257)
NT_W = [512] * 12 + [256]  # logits tile widths (12*512 + 256 = 6400 >= 6283)
VPAD = sum(NT_W)
RES_C = 5  # resident W_lin.T k-chunks (of 8); rest streamed per step
GATE_NT = 8  # 4096 gate dims / 512

F32 = mybir.dt.float32
F32R = mybir.dt.float32r
U32 = mybir.dt.uint32
I32 = mybir.dt.int32
AF = mybir.ActivationFunctionType
ALU = mybir.AluOpType

NEG = -1.0e30


def _round_f32r(a):
    """Bit-exact host replica of the device f32->f32r rounding (RNE, drop 12)."""
    u = np.ascontiguousarray(a, np.float32).view(np.uint32).astype(np.uint64)
    half = np.uint64(1 << 11)
    mask = np.uint64(0xFFFFF000)
    lsb = (u >> np.uint64(12)) & np.uint64(1)
    r = (u + half - np.uint64(1) + lsb) & mask
    return r.astype(np.uint32).view(np.float32)


def _legalize_multiwaits(nc):
    """This container's walrus accepts only one sync-wait per instruction.
    Hoist extra waits into standalone single-wait EventSemaphore instructions
    placed immediately before the owner (same engine => same program order)."""
    n = 0
    for f in nc.m.functions:
        for b in f.blocks:
            out = []
            changed = False
            for inst in b.instructions:
                si = inst.sync_info
                if si is not None and len(si.on_wait) > 1:
                    waits = list(si.on_wait)
                    for w in waits[:-1]:
                        n += 1
                        ev = mybir.InstEventSemaphore(
                            name=f"mwsplit-{n}", ins=[], outs=[]
                        )
                        ev.engine = inst.engine
                        ev.sync_info = mybir.SyncInfo(on_wait=[w], on_update=[])
                        out.append(ev)
                    si.on_wait = [waits[-1]]
                    inst.sync_info = si
                    changed = True
                out.append(inst)
            if changed:
                b.instructions = out
    return n


def _build():
    nc = bass.Bass(trn_type="TRN2", num_devices=NCORES)

    wlt_d = nc.dram_tensor("wlt", [8, 128, VPAD], F32R, kind="ExternalInput")
    blin_d = nc.dram_tensor("blin", [VPAD], F32R, kind="ExternalInput")
    whh_d = nc.dram_tensor("whh", [128, 4096], F32R, kind="ExternalInput")
    wih_d = nc.dram_tensor("wih", [64, 4096], F32R, kind="ExternalInput")
    biasg_d = nc.dram_tensor("biasg", [128, 32], F32, kind="ExternalInput")
    embx_d = nc.dram_tensor("embx", [V * 64, 1], F32, kind="ExternalInput")
    x0_d = nc.dram_tensor("x0", [64, 1], F32, kind="ExternalInput")
    cst_d = nc.dram_tensor("cst", [1, 32], F32, kind="ExternalInput")
    cstp_d = nc.dram_tensor("cstp", [128, 9], F32, kind="ExternalInput")
    out_d = nc.dram_tensor("out", [STEPS], F32, kind="ExternalOutput")
    dbg2 = os.environ.get("KDBG", "0") == "2"
    if dbg2:
        og_d = nc.dram_tensor("og", [STEPS], F32, kind="ExternalOutput")
        oe_d = nc.dram_tensor("oe", [STEPS, 16], F32, kind="ExternalOutput")
        om_d = nc.dram_tensor("om", [STEPS, 16], F32, kind="ExternalOutput")
        ot_d = nc.dram_tensor("ot", [STEPS], F32, kind="ExternalOutput")
    dbg = os.environ.get("KDBG", "0") == "1"
    if dbg:
        dbg_d = nc.dram_tensor("dbg", [128, 64], F32, kind="ExternalOutput")
        dbg2_d = nc.dram_tensor("dbg2", [1, 1152], F32, kind="ExternalOutput")
    pid_d = nc.partition_id_tensor

    with tile.TileContext(nc) as tc:
        with (
            tc.tile_pool(name="big", bufs=1) as big,
            tc.tile_pool(name="sm", bufs=1) as sm,
            tc.tile_pool(name="stream", bufs=6) as stream,
            tc.tile_pool(name="blinp", bufs=2) as blinp,
            tc.tile_pool(name="logp", bufs=3) as logp,
            tc.tile_pool(name="mxp", bufs=2) as mxp,
            tc.tile_pool(name="cell", bufs=2) as cellp,
            tc.tile_pool(name="gemv_ps", bufs=3, space="PSUM") as gemv_ps,
            tc.tile_pool(name="pre_ps", bufs=2, space="PSUM") as pre_ps,
            tc.tile_pool(name="dram", bufs=2, space="DRAM") as dram,
        ):
            # ---- resident weights
            wlt_sb = big.tile([128, RES_C * VPAD], F32R)
            for c in range(RES_C):
                nc.sync.dma_start(
                    wlt_sb[:, c * VPAD : (c + 1) * VPAD], wlt_d[c]
                )
            whh_sb = big.tile([128, 4096], F32R)
            nc.sync.dma_start(whh_sb[:], whh_d[:])
            wih_sb = big.tile([64, 4096], F32R)
            nc.sync.dma_start(wih_sb[:], wih_d[:])
            biasg_sb = sm.tile([128, 32], F32)
            nc.sync.dma_start(biasg_sb[:], biasg_d[:])

            # ---- constants / rank
            cst = sm.tile([1, 32], F32)
            nc.sync.dma_start(cst[:], cst_d[:])
            cstp = sm.tile([128, 9], F32)  # col0: arange(128); col1..8: iota8 rows
            nc.sync.dma_start(cstp[:], cstp_d[:])
            rank_u = sm.tile([128, 1], U32)
            nc.sync.dma_start(rank_u[:], pid_d[0:1, 0:1].to_broadcast([128, 1]))
            rankf = sm.tile([128, 1], F32)
            nc.vector.tensor_copy(rankf[:], rank_u[:])
            # rankmask[p, c] = (c == rank)
            rankmask = sm.tile([128, 8], F32)
            nc.vector.tensor_scalar(
                rankmask[:], cstp[:, 1:9], rankf[:], None, op0=ALU.is_equal
            )
            # base16 = 512*t + rank*VSH  (t = logits tile index)
            base16 = sm.tile([1, 16], F32)
            v0f = sm.tile([1, 1], F32)
            nc.vector.tensor_scalar(v0f[:], rankf[0:1, :], float(VSH), None, op0=ALU.mult)
            nc.vector.tensor_scalar(base16[:], cst[:, 0:16], v0f[:], None, op0=ALU.add)
            onec = sm.tile([1, 1], F32R)
            nc.vector.tensor_copy(onec[:], cst[:, 20:21])

            # ---- state
            x_r = sm.tile([64, 1], F32R)
            xg = sm.tile([64, 1], F32)
            nc.sync.dma_start(xg[:], x0_d[:])
            nc.vector.tensor_copy(x_r[:], xg[:])
            h_sb = sm.tile([128, 8], F32)
            c_sb = sm.tile([128, 8], F32)
            h_r = sm.tile([128, 8], F32R)
            hmy = sm.tile([128, 1], F32R)
            nc.vector.memset(h_sb[:], 0.0)
            nc.vector.memset(c_sb[:], 0.0)
            nc.vector.tensor_copy(h_r[:], h_sb[:])
            nc.vector.tensor_copy(hmy[:], h_sb[:, 0:1])

            colmax = sm.tile([1, 16], F32)
            colidxf = sm.tile([1, 16], F32)
            nc.vector.memset(colmax[:], NEG)
            nc.vector.memset(colidxf[:], 0.0)
            toks_f = sm.tile([128, max(STEPS, 8)], F32)
            gates_sb = sm.tile([128, 32], F32)
            gates2 = sm.tile([128, 32], F32)
            exch = sm.tile([64, 16], F32)
            if dbg:
                dbg2_sb = sm.tile([1, 1152], F32)
                nc.vector.memset(dbg2_sb[:], 0.0)

            for t in range(STEPS):
                # ---------- LSTM pre-activation partials ----------
                bB_in = dram.tile([4096], F32, tag="bBi")
                bB_out = dram.tile([4096], F32, tag="bBo")
                for g in range(GATE_NT):
                    ps = pre_ps.tile([1, 512], F32, tag="pre")
                    nc.tensor.matmul(
                        ps[:], x_r[:], wih_sb[:, g * 512 : (g + 1) * 512],
                        start=True, stop=False,
                    )
                    nc.tensor.matmul(
                        ps[:], hmy[:], whh_sb[:, g * 512 : (g + 1) * 512],
                        start=False, stop=True,
                    )
                    pse = logp.tile([1, 512], F32, tag="pse")
                    nc.scalar.activation(pse[:], ps[:], AF.Copy)
                    nc.sync.dma_start(bB_in[g * 512 : (g + 1) * 512], pse[0:1, :])
                nc.gpsimd.collective_compute(
                    "AllReduce", ALU.add,
                    replica_groups=[list(range(NCORES))],
                    ins=[bB_in.opt()], outs=[bB_out.opt()],
                )
                nc.sync.dma_start(
                    gates_sb[:], bB_out.rearrange("(c p) -> p c", p=128)
                )
                nc.vector.tensor_add(gates2[:], gates_sb[:], biasg_sb[:])

                # ---------- cell ----------
                sig_if = cellp.tile([128, 16], F32, tag="sif")
                tanh_g = cellp.tile([128, 8], F32, tag="tg")
                sig_o = cellp.tile([128, 8], F32, tag="so")
                nc.scalar.activation(sig_if[:], gates2[:, 0:16], AF.Sigmoid)
                nc.scalar.activation(tanh_g[:], gates2[:, 16:24], AF.Tanh)
                nc.scalar.activation(sig_o[:], gates2[:, 24:32], AF.Sigmoid)
                t1 = cellp.tile([128, 8], F32, tag="t1")
                t2 = cellp.tile([128, 8], F32, tag="t2")
                nc.vector.tensor_mul(t1[:], sig_if[:, 8:16], c_sb[:])
                nc.vector.tensor_mul(t2[:], sig_if[:, 0:8], tanh_g[:])
                nc.vector.tensor_add(c_sb[:], t1[:], t2[:])
                tanh_c = cellp.tile([128, 8], F32, tag="tc")
                nc.scalar.activation(tanh_c[:], c_sb[:], AF.Tanh)
                nc.vector.tensor_mul(h_sb[:], sig_o[:], tanh_c[:])
                nc.vector.tensor_copy(h_r[:], h_sb[:])
                # my h-chunk (column `rank`) for next step's W_hh partial
                hsel = cellp.tile([128, 8], F32, tag="hsel")
                nc.vector.tensor_mul(hsel[:], h_sb[:], rankmask[:])
                hmyf = cellp.tile([128, 1], F32, tag="hmyf")
                nc.vector.tensor_reduce(hmyf[:], hsel[:], mybir.AxisListType.X, ALU.add)
                nc.vector.tensor_copy(hmy[:], hmyf[:])

                # ---------- GEMV logits + per-tile top1 ----------
                off = 0
                for nt, w in enumerate(NT_W):
                    ps = gemv_ps.tile([1, 512], F32, tag="gv")
                    bl = blinp.tile([1, 512], F32R, tag="bl")
                    nc.sync.dma_start(bl[:, :w], blin_d[off : off + w])
                    nc.tensor.matmul(
                        ps[:, :w], onec[:], bl[:, :w], start=True, stop=False
                    )
                    for c in range(RES_C):
                        nc.tensor.matmul(
                            ps[:, :w], h_r[:, c : c + 1],
                            wlt_sb[:, c * VPAD + off : c * VPAD + off + w],
                            start=False, stop=False,
                        )
                    for c in range(RES_C, 8):
                        st = stream.tile([128, 512], F32R, tag="st")
                        nc.sync.dma_start(st[:, :w], wlt_d[c][:, off : off + w])
                        nc.tensor.matmul(
                            ps[:, :w], h_r[:, c : c + 1], st[:, :w],
                            start=False, stop=(c == 7),
                        )
                    lg = logp.tile([1, 512], F32, tag="lg")
                    nc.scalar.activation(lg[:, :w], ps[:, :w], AF.Copy)
                    if dbg and t == 1 and nt in (2, 4):
                        nc.vector.tensor_copy(
                            dbg2_sb[0:1, 512 * (nt // 2 - 1) : 512 * (nt // 2)],
                            lg[:, :512],
                        )
                    mx = mxp.tile([1, 8], F32, tag="mx")
                    ix = mxp.tile([1, 8], U32, tag="ix")
                    ixf = mxp.tile([1, 8], F32, tag="ixf")
                    nc.vector.max(mx[:], lg[:, :w])
                    nc.vector.max_index(ix[:], mx[:], lg[:, :w])
                    nc.vector.tensor_copy(ixf[:], ix[:])
                    nc.vector.tensor_copy(colmax[:, nt : nt + 1], mx[:, 0:1])
                    nc.vector.tensor_copy(colidxf[:, nt : nt + 1], ixf[:, 0:1])
                    off += w

                # ---------- local winner ----------
                gmax8 = mxp.tile([1, 8], F32, tag="gm8")
                nc.vector.max(gmax8[:], colmax[:])
                m16 = mxp.tile([1, 16], F32, tag="m16")
                nc.vector.tensor_scalar(
                    m16[:], colmax[:], gmax8[:, 0:1], None, op0=ALU.is_equal
                )
                offs16 = mxp.tile([1, 16], F32, tag="o16")
                nc.vector.tensor_add(offs16[:], colidxf[:], base16[:])
                cand = mxp.tile([1, 16], F32, tag="cand")
                nc.vector.tensor_mul(cand[:], offs16[:], m16[:])
                gidx = mxp.tile([1, 1], F32, tag="gidx")
                nc.vector.tensor_reduce(gidx[:], cand[:], mybir.AxisListType.X, ALU.max)

                # ---------- argmax exchange ----------
                bA_in = dram.tile([2], F32, tag="bAi")
                bA_out = dram.tile([16], F32, tag="bAo")
                nc.sync.dma_start(bA_in[0:1], gmax8[0:1, 0:1])
                nc.sync.dma_start(bA_in[1:2], gidx[0:1, 0:1])
                nc.gpsimd.collective_compute(
                    "AllGather", ALU.bypass,
                    replica_groups=[list(range(NCORES))],
                    ins=[bA_in.opt()], outs=[bA_out.opt()],
                )
                nc.sync.dma_start(exch[:], bA_out[None, :].to_broadcast([64, 16]))
                wm8 = mxp.tile([64, 8], F32, tag="wm8")
                nc.vector.max(wm8[:], exch[:, 0:16:2])
                wmask = mxp.tile([64, 8], F32, tag="wmask")
                nc.vector.tensor_scalar(
                    wmask[:], exch[:, 0:16:2], wm8[:, 0:1], None, op0=ALU.is_equal
                )
                wcand = mxp.tile([64, 8], F32, tag="wcand")
                nc.vector.tensor_mul(wcand[:], exch[:, 1:16:2], wmask[:])
                tokf = mxp.tile([64, 1], F32, tag="tokf")
                nc.vector.tensor_reduce(tokf[:], wcand[:], mybir.AxisListType.X, ALU.max)
                nc.vector.tensor_copy(toks_f[0:64, t : t + 1], tokf[:])

                if dbg2:
                    nc.sync.dma_start(og_d[t : t + 1], gidx[0:1, :])
                    nc.sync.dma_start(oe_d[t], exch[0:1, :])
                    nc.sync.dma_start(om_d[t], colmax[0:1, :])
                    nc.sync.dma_start(ot_d[t : t + 1], tokf[0:1, 0:1])

                # ---------- next x = emb[tok] (own 64-dim slice) ----------
                if t + 1 < STEPS:
                    offp = mxp.tile([64, 1], F32, tag="offp")
                    nc.vector.scalar_tensor_tensor(
                        offp[:], tokf[0:64, :], 64.0, cstp[0:64, 0:1],
                        op0=ALU.mult, op1=ALU.add,
                    )
                    offu = mxp.tile([64, 1], U32, tag="offu")
                    nc.vector.tensor_copy(offu[:], offp[:])
                    nc.gpsimd.indirect_dma_start(
                        xg[:], None, embx_d[:],
                        IndirectOffsetOnAxis(ap=offu[:], axis=0),
                    )
                    nc.vector.tensor_copy(x_r[:], xg[:])

                if dbg and t == 1:
                    dbg_sb = sm.tile([128, 64], F32)
                    nc.vector.memset(dbg_sb[:], 0.0)
                    nc.vector.tensor_copy(dbg2_sb[0:1, 1024:1040], exch[0:1, :])
                    nc.vector.tensor_copy(dbg2_sb[0:1, 1040:1048], wm8[0:1, :])
                    nc.vector.tensor_copy(dbg2_sb[0:1, 1048:1056], wmask[0:1, :])
                    nc.vector.tensor_copy(dbg2_sb[0:1, 1056:1064], wcand[0:1, :])
                    nc.vector.tensor_copy(dbg2_sb[0:1, 1064:1080], offs16[:])
                    nc.vector.tensor_copy(dbg2_sb[0:1, 1080:1096], m16[:])
                    nc.vector.tensor_copy(dbg2_sb[0:1, 1096:1112], colidxf[:])
                    nc.vector.tensor_copy(dbg2_sb[0:1, 1112:1128], base16[:])
                    nc.vector.tensor_copy(dbg_sb[:, 0:8], h_r[:].bitcast(F32))
                    nc.vector.tensor_copy(dbg_sb[:, 8:9], hmy[:].bitcast(F32))
                    nc.vector.tensor_copy(dbg_sb[:, 9:41], gates2[:])
                    nc.vector.tensor_copy(dbg_sb[0:64, 41:42], xg[:])
                    nc.vector.tensor_copy(
                        dbg_sb[0:64, 42:43], x_r[:].bitcast(F32)
                    )
                    nc.vector.tensor_copy(dbg_sb[0:64, 43:44], tokf[:])
                    nc.vector.tensor_copy(dbg_sb[0:1, 44:45], gidx[:])
                    nc.vector.tensor_copy(dbg_sb[0:1, 45:46], gmax8[:, 0:1])
                    nc.vector.tensor_copy(dbg_sb[0:1, 46:62], colmax[:])
                    nc.vector.tensor_copy(dbg_sb[0:1, 62:63], colidxf[:, 2:3])
                    nc.vector.tensor_copy(dbg_sb[0:1, 63:64], colidxf[:, 4:5])
                    nc.sync.dma_start(dbg_d[:], dbg_sb[:])

            if dbg:
                nc.sync.dma_start(dbg2_d[:], dbg2_sb[:])
            nc.sync.dma_start(out_d[:], toks_f[0:1, 0:STEPS])

    _legalize_multiwaits(nc)
    return nc


def _prep_inputs(inp, W_ih, W_hh, b_ih, b_hh, emb, W_lin, b_lin):
    in_maps = []
    bias = (b_ih + b_hh).astype(np.float32)
    biasg = bias.reshape(32, 128).T.copy()  # [128, 32]: (p, c) = bias[128c+p]
    cst = np.zeros((1, 32), np.float32)
    cst[0, 0:16] = 512.0 * np.arange(16)
    cst[0, 20] = 1.0
    cstp = np.zeros((128, 9), np.float32)
    cstp[:, 0] = np.arange(128)
    cstp[:, 1:9] = np.arange(8)[None, :]
    for k in range(NCORES):
        lo = k * VSH
        hi = min(V, lo + VSH)
        nrows = hi - lo
        Wk = np.zeros((VPAD, H), np.float32)
        Wk[:nrows] = W_lin[lo:hi]
        bk = np.full(VPAD, NEG, np.float32)
        bk[:nrows] = b_lin[lo:hi]
        wlt = np.ascontiguousarray(
            _round_f32r(Wk).T.reshape(8, 128, VPAD)
        )  # [c, p, v] = Wk[v, 128c+p]
        whh = _round_f32r(W_hh[:, 128 * k : 128 * (k + 1)].T.copy())  # [128, 4096]
        wih = _round_f32r(W_ih[:, 64 * k : 64 * (k + 1)].T.copy())  # [64, 4096]
        embx = np.ascontiguousarray(
            emb[:, 64 * k : 64 * (k + 1)].astype(np.float32).reshape(V * 64, 1)
        )
        x0 = np.ascontiguousarray(
            inp[0, 0, 64 * k : 64 * (k + 1)].astype(np.float32).reshape(64, 1)
        )
        in_maps.append(
            {
                "wlt": wlt,
                "blin": _round_f32r(bk),
                "whh": whh,
                "wih": wih,
                "biasg": biasg,
                "embx": embx,
                "x0": x0,
                "cst": cst,
                "cstp": cstp,
            }
        )
    return in_maps


_CACHE = {}


def kernel(**inputs):
    inputs = {k: np.asarray(v, np.float32) for k, v in inputs.items()}
    if "nc" not in _CACHE:
        _CACHE["nc"] = _build()
    nc = _CACHE["nc"]
    in_maps = _prep_inputs(**inputs)
    res = run_bass_kernel_spmd(nc, in_maps, core_ids=list(range(NCORES)))
    _CACHE["res"] = res
    return np.rint(res.results[0]["out"]).astype(np.int32)

